# revision 10
# baseline (speedup 1.0000x reference)
"""Trainium2 Bass kernel for nn_Bilinear_86328842650062.

Computes out[s,i,j] = sum_{d,e} tensor1[s,i,d] * W[d,e] * tensor0[s,j,e] + bias
for S=4, N=4096, D=64, then tiles to batch 2:  output (2, 4, 4096, 4096) f32.

Strategy (classic 1D row-parallel): shard the i axis (rows of tensor1 /
rows of the output) across 8 NeuronCores, 512 rows each; replicate the
small (D,D) kernel and tensor0.  Per core and per s:
    B[s] = x1_shard[s] @ W            (512x64 @ 64x64, one f32r matmul)
    out_shard[s] = B[s] @ x0[s]^T     (512x64 @ 64x4096 per s)
Host-side we pre-transpose x0/x1 so the contraction dim (64) lands on
SBUF partitions, packing two s-slices per 128 partitions (s even ->
partitions 0-63, s odd -> 64-127).  The batch-2 leading dim is a pure
broadcast and is materialized host-side as a stride-0 view.

Perf structure: the kernel is HBM-write-bound (16.8 MB of f16 output
per core vs ~350 GB/s per-core HBM).  The big matmuls for an (s-even,
s-odd) pair sit in disjoint PE row groups (h0 / h64, contraction = 64),
so interleaving them runs both CONCURRENTLY on the PE array (2x matmul
throughput; the PE clock sits at the throttled 1.2 GHz rate all run).
PSUM->SBUF copies split between DVE (s even) and ACT (s odd) run
concurrently too, so 2 MB output blocks are produced at ~460 GB/s and
the sync-ring output DMA stays saturated from ~6 us onward.
"""

import os as _os

import numpy as np

S, N, D = 4, 4096, 64
N_CORES = 8
ROWS = N // N_CORES  # 512 output rows per core
BATCH = 2

BIG_DT = _os.environ.get("BASS_BIG_DT", "float16")  # "float16" | "bfloat16"
OUT_DT = _os.environ.get("BASS_OUT_DT", "float16")

_CACHE = {}


def _build(big_dt_name, out_dt_name):
    import concourse.bacc as bacc
    import concourse.tile as tile
    import concourse.mybir as mybir

    dt = mybir.dt
    f32 = dt.float32
    f16 = getattr(dt, big_dt_name)
    out_dt = getattr(dt, out_dt_name)

    nc = bacc.Bacc(
        "TRN2",
        target_bir_lowering=False,
        debug=False,
        enable_asserts=False,
        num_devices=N_CORES,
    )
    # DRAM I/O. wx1 packs [W | x1t(a=0) | x1t(a=1)] partition-major
    # (p = 64*(s%2)+d) into one 16-bit array so a single 278 KB DMA
    # feeds the small matmuls; the ~2e-4 rounding of W/x1 is buried
    # under the 16-bit rounding of B that follows.
    wx1_dram = nc.dram_tensor(
        "wx1", [128, D + (S // 2) * ROWS], f16, kind="ExternalInput"
    ).ap()
    x0h_dram = nc.dram_tensor("x0h", [S, D, N], f16, kind="ExternalInput").ap()
    out_dram = nc.dram_tensor("out", [S, ROWS, N], out_dt, kind="ExternalOutput").ap()

    IT = ROWS // 128  # 4 row-tiles of 128 output rows per s
    JT2 = N // 1024   # 4 psum col-blocks (2 banks each) per row-tile

    with tile.TileContext(nc) as tc:
        with (
            tc.tile_pool(name="const", bufs=1) as const_pool,
            tc.tile_pool(name="outsb", bufs=3) as out_pool,
            tc.tile_pool(name="pse", bufs=2, space="PSUM") as pse_pool,
            tc.tile_pool(name="pso", bufs=2, space="PSUM") as pso_pool,
        ):
            # (S, D, X) -> sbuf [128, S//2, X]: partition p = 64*(s%2)+d,
            # free a = s//2.  In DRAM, (s, d) flattens to p-major order
            # (a p) since stride(s) = D*X and stride(d) = X.  Loads spread
            # over three rings so their fixed latencies overlap: sync
            # carries what the small matmuls need (W, x1t), scalar/gpsimd
            # carry the two x0 halves (a=0 / a=1).
            wx1_sb = const_pool.tile([128, D + (S // 2) * ROWS], f16)
            wt = wx1_sb[:, 0:D]
            x0h_sb = const_pool.tile([128, S // 2, N], f16)
            x0h_r = x0h_dram.rearrange("(a ps) d x -> (ps d) a x", ps=2)
            # B^T for all four s, 16-bit: [p = 64*(s%2)+d, a, i]
            bt_sb = const_pool.tile([128, S // 2, ROWS], f16)

            # wx1 in two pieces so phase 1 for s=0,1 starts after only
            # W + x1(a=0) (147 KB) lands; x0 in 256 KB quarter-chunks
            # interleaved across the scalar/gpsimd rings, a=0 first, so
            # the first big matmuls (cols 0-1023 of a=0) unblock on a
            # minimal prefix of the input traffic.
            nc.sync.dma_start(wx1_sb[:, 0 : D + ROWS], wx1_dram[:, 0 : D + ROWS])
            nc.sync.dma_start(wx1_sb[:, D + ROWS :], wx1_dram[:, D + ROWS :])
            Q = N // 4
            for a in range(S // 2):
                for c in range(4):
                    eng = nc.scalar if c % 2 == 0 else nc.gpsimd
                    qsl = slice(c * Q, (c + 1) * Q)
                    eng.dma_start(x0h_sb[:, a, qsl], x0h_r[:, a, qsl])

            # Phase 1: B^T[s] = (x1[s] @ W)^T via 16-bit matmuls,
            # psum[e,i] = sum_d W[d,e] x1t[d,i].  s-even uses PE rows
            # 0-63, s-odd rows 64-127 -> each pair runs concurrently.
            # PSUM comes from the big-loop pools (no extra banks); the
            # 16-bit rounding copy splits DVE (even) / ACT (odd).
            for s in range(S):
                p0 = (s % 2) * D
                a = s // 2
                if s % 2 == 0:
                    ps_e = pse_pool.tile([128, 1024], f32)
                    ps_b = ps_e[0:D, 0:ROWS]
                else:
                    ps_o = pso_pool.tile([128, 1024], f32)
                    ps_b = ps_o[0:D, 0:ROWS]
                nc.tensor.matmul(
                    ps_b,
                    wt[p0 : p0 + D, :],
                    wx1_sb[p0 : p0 + D, D + a * ROWS : D + (a + 1) * ROWS],
                    start=True,
                    stop=True,
                )
                if s % 2 == 0:
                    nc.vector.tensor_copy(bt_sb[p0 : p0 + D, a, :], ps_b)
                else:
                    nc.scalar.copy(bt_sb[p0 : p0 + D, a, :], ps_b)

            # Phase 2: out_shard[s] = B[s] @ x0[s]^T, s-pairs interleaved.
            # Per (a, it): 16 matmuls in 8 concurrent h0/h64 pairs feed
            # two [128, 4096] output tiles; per jt2 the two PSUM drains
            # run in lockstep on DVE (s even) || ACT (s odd) - anything
            # fancier (greedy balancing) breaks the 2-deep PSUM rotation
            # and stalls the PE.  Each finished 2048-col half drains on
            # the sync HWDGE ring (1024-col granularity on the first
            # block only, to start the output stream sooner; small DMAs
            # at the END drain slower, so the last block stays coarse).
            for a in range(S // 2):
                for it in range(IT):
                    isl = slice(it * 128, (it + 1) * 128)
                    out_e = out_pool.tile([128, N], out_dt)
                    out_o = out_pool.tile([128, N], out_dt)
                    first_blk = a == 0 and it == 0
                    for jt2 in range(JT2):
                        ps_e = pse_pool.tile([128, 1024], f32)
                        ps_o = pso_pool.tile([128, 1024], f32)
                        for h in range(2):
                            jsl = slice((jt2 * 2 + h) * 512, (jt2 * 2 + h + 1) * 512)
                            hsl = slice(h * 512, (h + 1) * 512)
                            nc.tensor.matmul(
                                ps_e[:, hsl],
                                bt_sb[0:D, a, isl],
                                x0h_sb[0:D, a, jsl],
                                start=True,
                                stop=True,
                            )
                            nc.tensor.matmul(
                                ps_o[:, hsl],
                                bt_sb[D : 2 * D, a, isl],
                                x0h_sb[D : 2 * D, a, jsl],
                                start=True,
                                stop=True,
                            )
                        csl = slice(jt2 * 1024, (jt2 + 1) * 1024)
                        nc.vector.tensor_copy(out_e[:, csl], ps_e[:])
                        nc.scalar.copy(out_o[:, csl], ps_o[:])
                        if first_blk:
                            nsl = csl
                        elif jt2 % 2 == 1:
                            nsl = slice((jt2 - 1) * 1024, (jt2 + 1) * 1024)
                        else:
                            continue
                        nc.sync.dma_start(out_dram[2 * a, isl, nsl], out_e[:, nsl])
                        nc.sync.dma_start(
                            out_dram[2 * a + 1, isl, nsl], out_o[:, nsl]
                        )
    nc.compile()
    return nc


def _get_nc():
    key = (BIG_DT, OUT_DT)
    if key not in _CACHE:
        _CACHE[key] = _build(BIG_DT, OUT_DT)
    return _CACHE[key]


LAST_RESULTS = None


def kernel(**inputs):
    from concourse.bass_utils import run_bass_kernel_spmd

    global LAST_RESULTS

    tensor0 = np.ascontiguousarray(np.asarray(inputs["tensor0"], dtype=np.float32))
    tensor1 = np.ascontiguousarray(np.asarray(inputs["tensor1"], dtype=np.float32))
    W = np.ascontiguousarray(np.asarray(inputs["kernel"], dtype=np.float32))
    bias = float(np.asarray(inputs["bias"]))

    # Host prep: contraction dim to axis -2 for partition-major DMA,
    # 16-bit cast of x0/x1/W.
    if BIG_DT == "float16":
        big_np = np.float16
    else:
        import ml_dtypes

        big_np = ml_dtypes.bfloat16
    x0t = np.ascontiguousarray(tensor0.transpose(0, 2, 1))  # (S, D, N)
    x0h = x0t.astype(big_np)
    x1t_full = tensor1.transpose(0, 2, 1)  # (S, D, N) view

    in_maps = []
    for c in range(N_CORES):
        # Pack [W | x1t(a=0) | x1t(a=1)] partition-major (p = 64*(s%2)+d)
        # into one array so a single DMA feeds the small matmuls.
        x1c = x1t_full[:, :, c * ROWS : (c + 1) * ROWS]  # (S, D, ROWS)
        wx1 = np.empty((128, D + (S // 2) * ROWS), dtype=big_np)
        wx1[0:D, 0:D] = W
        wx1[D : 2 * D, 0:D] = W
        for a in range(S // 2):
            csl = slice(D + a * ROWS, D + (a + 1) * ROWS)
            wx1[0:D, csl] = x1c[2 * a]
            wx1[D : 2 * D, csl] = x1c[2 * a + 1]
        in_maps.append({"wx1": wx1, "x0h": x0h})

    nc = _get_nc()
    res = run_bass_kernel_spmd(nc, in_maps, list(range(N_CORES)))
    LAST_RESULTS = res

    out_full = np.empty((S, N, N), dtype=np.float32)
    for c in range(N_CORES):
        out_full[:, c * ROWS : (c + 1) * ROWS, :] = res.results[c]["out"].astype(
            np.float32, copy=False
        )

    if bias != 0.0:
        out_full += np.float32(bias)

    return np.broadcast_to(out_full[None], (BATCH, S, N, N))
